# revision 22
# baseline (speedup 1.0000x reference)
"""Trainium2 Bass kernel for nn_FIN_b: windowed-FM tabular net.

Data-parallel over batch: B=2048 rows split across 8 NeuronCores (256 each).
Activations are feature-major ([feature_partition, batch_free]); every matmul
uses natural-layout weights as the stationary operand.  The windowed FM block
    fm_out[b,c] = 0.5*(sum_e (sum_f x[b,c+f] v[c,f,e])^2 - sum_f x^2 v^2)
is algebraically reduced (f==f' diagonal cancels) to
    fm_out[b,c] = sum_{d=1..7} sum_f D_d[b, c+f] * G[c, f, f+d],
    D_d = x * shift_d(x),  G[c,f,g] = sum_e v[c,f,e] v[c,g,e],
i.e. 7 shifted elementwise products followed by a banded contraction on the
tensor engine with host-precomputed block-banded weights.  The FM linear term
(x_fm @ lin_w) is folded into W1's top half on the host.

Scheduling: ALL bulk data streams on the sync-engine HWDGE ring (measured
the only fast DMA path here: ~230-320 GB/s vs ~20-130 GB/s for the scalar
ring / SWDGE), issued in consumption order with the seven partition-shift
copies spliced into the FIFO between the two W1a halves.  The FM product
path runs in scaled fp8e4m3 (x8/xs/D/G band; G and W1b scaled x64 against
e4m3's subnormal floor), halving both shift traffic and G/W1b weight bytes;
W1a stays bf16 (the x path carries too much signal for fp8; fp8 there
measures 2.2e-2 rel err vs the 2e-2 budget).  Matmuls mix operand dtypes
freely (fp8 stationary x bf16 moving verified exact on HW).  The 8
h-accumulator regions pack 2-per-PSUM-bank (banks pre-zeroed by DVE memset
so every W1 matmul runs start=False: accumulate-onto-zero is correct for
any stale has_written state), freeing 4 banks for the FM reductions, which
interleave with the W1b k-stream on the PE.  A burst of dummy matmuls at
t=0 warms the PE HAM clock gate during the DMA prologue; front relu is
split ACT/DVE and the h-lrelu tail ACT/DVE so no single engine serializes
an epilogue.
"""

import sys

sys.path.insert(0, "/opt/trn_rl_repo")

import numpy as np
import ml_dtypes

import concourse.bass as bass
import concourse.tile as tile
from concourse import bacc, mybir
from concourse.bass_utils import run_bass_kernel_spmd

NDF, NCF, NCC = 512, 256, 256
EMB, FIELD = 16, 8
B = 2048
NH0 = NDF + 2 * NCC          # 1024
CHANNEL = NH0 - FIELD + 1    # 1017
HID = (NH0 + CHANNEL) // 2   # 1020
NCORES = 8
BC = B // NCORES             # 256 batch rows per core

F32 = mybir.dt.float32
BF16 = mybir.dt.bfloat16
FP8 = mybir.dt.float8e4

GSCALE = 64.0                # lifts G (~1e-2) and W1b out of e4m3 subnormals

BF = ml_dtypes.bfloat16
E4 = ml_dtypes.float8_e4m3fn

_cache = {}


def _build(b2_val: float):
    nc = bacc.Bacc()

    xdT_d = nc.dram_tensor("xdT", [128, 4, BC], BF16, kind="ExternalInput")
    xcT_d = nc.dram_tensor("xcT", [128, 2, BC], BF16, kind="ExternalInput")
    Wd_d = nc.dram_tensor("Wd", [128, 4, NDF], BF16, kind="ExternalInput")
    Wc_d = nc.dram_tensor("Wc", [128, 2, 2 * NCC], BF16, kind="ExternalInput")
    bias_d = nc.dram_tensor("bias", [128, 16], F32, kind="ExternalInput")
    Gm_d = nc.dram_tensor("Gm", [128, 7, 8, 128], FP8, kind="ExternalInput")
    Gs_d = nc.dram_tensor("Gs", [36, 7, 128], FP8, kind="ExternalInput")
    W1a_d = nc.dram_tensor("W1a", [128, 8, 1024], BF16, kind="ExternalInput")
    W1b_d = nc.dram_tensor("W1b", [128, 8, 1024], FP8, kind="ExternalInput")
    W2_d = nc.dram_tensor("W2", [128, 8, 1], BF16, kind="ExternalInput")
    out_d = nc.dram_tensor("out", [1, BC], F32, kind="ExternalOutput")

    AF = mybir.ActivationFunctionType

    with tile.TileContext(nc) as tc:
        with (
            tc.tile_pool(name="w", bufs=1) as wp,
            tc.tile_pool(name="act", bufs=1) as ap,
            tc.tile_pool(name="hp", bufs=4, space=bass.MemorySpace.PSUM) as hp,
            tc.tile_pool(name="rp", bufs=4, space=bass.MemorySpace.PSUM) as rp,
        ):
            # ---- weight/input DMAs on the sync HWDGE ring, in
            #      consumption (priority) order ----
            # Everything bulk rides the sync HWDGE ring (the only fast DMA
            # path: ~320 GB/s chunked; the scalar ring and SWDGE measure
            # 5-15x slower).  Issue order = FIFO order = consumption order.
            xdT = wp.tile([128, 4, BC], BF16, tag="xdT")
            nc.sync.dma_start(xdT[:], xdT_d[:])
            Wd = wp.tile([128, 4, NDF], BF16, tag="Wd")
            nc.sync.dma_start(Wd[:], Wd_d[:])
            bias = wp.tile([128, 16], F32, tag="bias")
            nc.sync.dma_start(bias[:], bias_d[:])
            xcT = wp.tile([128, 2, BC], BF16, tag="xcT")
            nc.sync.dma_start(xcT[:], xcT_d[:])
            Wc = wp.tile([128, 2, 2 * NCC], BF16, tag="Wc")
            nc.sync.dma_start(Wc[:], Wc_d[:])
            W1a = wp.tile([128, 8, 1024], BF16, tag="W1a")
            nc.sync.dma_start(W1a[:, 0:4, :], W1a_d[:, 0:4, :])
            Gm = wp.tile([128, 7, 8, 128], FP8, tag="Gm")
            nc.sync.dma_start(Gm[:], Gm_d[:])
            Gs = wp.tile([36, 7, 128], FP8, tag="Gs")
            nc.sync.dma_start(Gs[:], Gs_d[:])
            W2 = wp.tile([128, 8, 1], BF16, tag="W2")
            nc.sync.dma_start(W2[:], W2_d[:])
            W1b = wp.tile([128, 8, 1024], FP8, tag="W1b")

            x = ap.tile([128, 8, BC], BF16, tag="x")
            x8 = ap.tile([128, 9, BC], FP8, tag="x8")   # block 8 stays zero
            xst = [ap.tile([128, 8, BC], FP8, tag=f"xs{d}", name=f"xs{d}")
                   for d in range(1, 8)]
            D = ap.tile([128, 7, 8, BC], FP8, tag="D")
            aux = ap.tile([36, 7, BC], FP8, tag="aux")
            fm1 = ap.tile([128, 8, BC], BF16, tag="fm1")
            fm2 = ap.tile([128, 8, BC], BF16, tag="fm2")
            h = ap.tile([128, 8, BC], BF16, tag="h")
            sig = ap.tile([1, BC], F32, tag="sig")
            zer = ap.tile([8, 512], BF16, tag="zer")
            with tc.high_priority():
                nc.vector.memset(zer[:], 0.0)
            nc.vector.memset(x8[:, 8, :], 0.0)

            # h accumulators: 8 regions packed 2-per-bank.  Pre-zero the
            # banks so every W1 matmul can run start=False (accumulate onto
            # zeros is exact regardless of stale has_written bits).
            hb = [hp.tile([128, 512], F32, tag="hb", name=f"hb{j}")
                  for j in range(4)]
            for j in range(4):
                nc.vector.memset(hb[j][:], 0.0)

            def hreg(mt):
                j, r = mt % 4, mt // 4
                return hb[j][:, r * BC:(r + 1) * BC]

            # ---- HAM warmup: dummy matmuls keep the PE busy through the
            #      DMA prologue so the real work runs at 2.4 GHz ----
            wps = rp.tile([128, 512], F32, tag="rps", name="warm")
            with tc.high_priority():
                for i in range(8):
                    nc.tensor.matmul(
                        wps[:, 0:512], zer[0:8, 0:128], zer[0:8, 0:512],
                        start=True, stop=True,
                    )

            # ---- front: x = relu([Xd,Xc] @ [Wd,Wc] + b), feature-major ----
            for mt in range(4):
                ps = rp.tile([128, 512], F32, tag="rps", name=f"fpsd{mt}")
                for kt in range(4):
                    nc.tensor.matmul(
                        ps[:, 0:BC], Wd[:, kt, mt * 128:(mt + 1) * 128],
                        xdT[:, kt, :], start=(kt == 0), stop=(kt == 3),
                    )
                if mt % 2 == 0:
                    nc.scalar.activation(
                        out=x[:, mt, :], in_=ps[:, 0:BC], func=AF.Relu,
                        bias=bias[:, mt:mt + 1], scale=1.0,
                    )
                else:
                    nc.vector.tensor_scalar(
                        x[:, mt, :], ps[:, 0:BC], bias[:, mt:mt + 1], 0.0,
                        mybir.AluOpType.add, mybir.AluOpType.max,
                    )
                nc.vector.tensor_copy(x8[:, mt, :], x[:, mt, :])
            for mt in range(4):
                ps = rp.tile([128, 512], F32, tag="rps", name=f"fpsc{mt}")
                for kt in range(2):
                    nc.tensor.matmul(
                        ps[:, 0:BC], Wc[:, kt, mt * 128:(mt + 1) * 128],
                        xcT[:, kt, :], start=(kt == 0), stop=(kt == 1),
                    )
                if mt % 2 == 0:
                    nc.scalar.activation(
                        out=x[:, 4 + mt, :], in_=ps[:, 0:BC], func=AF.Relu,
                        bias=bias[:, 4 + mt:5 + mt], scale=1.0,
                    )
                else:
                    nc.vector.tensor_scalar(
                        x[:, 4 + mt, :], ps[:, 0:BC], bias[:, 4 + mt:5 + mt],
                        0.0, mybir.AluOpType.add, mybir.AluOpType.max,
                    )
                nc.vector.tensor_copy(x8[:, 4 + mt, :], x[:, 4 + mt, :])

            # ---- FM products: D_d = x * shift_d(x) in scaled fp8 ----
            # Shift mains are spliced into the sync FIFO right after W1a's
            # first half; tiny block-boundary pieces go on the scalar ring.
            for d in range(1, 8):
                xs = xst[d - 1]
                nc.sync.dma_start(xs[0:128 - d, :, :], x8[d:128, 0:8, :])
                nc.sync.dma_start(xs[128 - d:128, :, :], x8[0:d, 1:9, :])
                # split each product between DVE and gpsimd
                nc.vector.tensor_mul(
                    D[:, d - 1, 0:5, :], x8[:, 0:5, :], xs[:, 0:5, :]
                )
                nc.gpsimd.tensor_mul(
                    D[:, d - 1, 5:8, :], x8[:, 5:8, :], xs[:, 5:8, :]
                )
            # rest of the weight stream, behind the shifts in the FIFO;
            # W1b first (needed by the two-pass W1b stream), W1a's tail last
            # (its h contributions commute, so those matmuls run at the end)
            for ch in range(2):
                nc.sync.dma_start(
                    W1b[:, 4 * ch:4 * ch + 4, :], W1b_d[:, 4 * ch:4 * ch + 4, :]
                )
            nc.sync.dma_start(W1a[:, 4:8, :], W1a_d[:, 4:8, :])
            for d in range(1, 7):
                nc.sync.dma_start(
                    aux[6 * (d - 1):6 * d, :, :], D[0:6, d - 1, 1:8, :]
                )

            # ---- W1a: h += x-part; kt 4-7 run at the very end (their DMA
            #      lands last and h accumulation is order-free) ----
            for kt in range(4):
                for mt in range(8):
                    nc.tensor.matmul(
                        hreg(mt), W1a[:, kt, mt * 128:(mt + 1) * 128],
                        x[:, kt, :], start=False, stop=False,
                        skip_group_check=True,
                    )

            # ---- FM banded reductions, two passes: pass 1 accumulates the
            #      d=1..4 terms (available early), pass 2 the d=5..7 terms +
            #      block-boundary stragglers.  W1b is applied to each partial
            #      (h accumulation commutes), overlapping the product chain.
            def fm_pass(blocks, fmdst, drange, last_pass):
                pss = {}
                for Bb in blocks:
                    pss[Bb] = rp.tile([128, 512], F32, tag="rps",
                                      name=f"fmp{drange[0]}_{Bb}")
                for d in drange:
                    for Bb in blocks:
                        nc.tensor.matmul(
                            pss[Bb][:, 0:BC], Gm[:, d - 1, Bb, :],
                            D[:, d - 1, Bb, :], start=(d == drange[0]),
                            stop=(not last_pass and d == drange[-1])
                                 or (last_pass and d == 7 and Bb == 7),
                        )
                for Bb in blocks:
                    if last_pass and Bb < 7:
                        nc.tensor.matmul(
                            pss[Bb][:, 0:BC], Gs[:, Bb, :], aux[:, Bb, :],
                            start=False, stop=True,
                        )
                    # fm stored as true_fm/GSCALE (W1b carries the x64);
                    # copies on ACT so the DVE stays on the product chain
                    nc.scalar.activation(
                        out=fmdst[:, Bb, :], in_=pss[Bb][:, 0:BC],
                        func=AF.Copy, bias=0.0, scale=1.0 / (GSCALE * GSCALE),
                    )

            def w1b_ktile(kt, fmsrc):
                for mt in range(8):
                    nc.tensor.matmul(
                        hreg(mt), W1b[:, kt, mt * 128:(mt + 1) * 128],
                        fmsrc[:, kt, :], start=False, stop=False,
                        skip_group_check=True,
                    )

            fm_pass([0, 1, 2, 3], fm1, range(1, 5), False)
            fm_pass([4, 5, 6, 7], fm1, range(1, 5), False)
            for kt in range(8):
                w1b_ktile(kt, fm1)
            fm_pass([0, 1, 2, 3], fm2, range(5, 8), True)
            fm_pass([4, 5, 6, 7], fm2, range(5, 8), True)
            for kt in range(8):
                w1b_ktile(kt, fm2)

            # W1a k-tiles 4..7 close out the h accumulation (final k-tile in
            # bank-pair order so each bank frees for its lrelu early)
            for kt in range(4, 8):
                mts = [0, 4, 1, 5, 2, 6, 3, 7] if kt == 7 else range(8)
                for mt in mts:
                    nc.tensor.matmul(
                        hreg(mt), W1a[:, kt, mt * 128:(mt + 1) * 128],
                        x[:, kt, :], start=False, stop=(kt == 7),
                        skip_group_check=True,
                    )

            # ---- h = lrelu(hacc + b1); pred = sigmoid(h @ W2 + b2) ----
            ps1 = rp.tile([128, 512], F32, tag="rps", name="ps1")
            ht = ap.tile([128, 4, BC], F32, tag="ht")
            for i, mt in enumerate([0, 4, 1, 5, 2, 6, 3, 7]):
                if mt in (0, 4, 1, 5):
                    nc.scalar.activation(
                        out=h[:, mt, :], in_=hreg(mt), func=AF.Lrelu,
                        bias=bias[:, 8 + mt:9 + mt], scale=1.0, alpha=0.01,
                    )
                else:
                    # DVE lrelu: t = z + b1; h = max(0.01*t, t)
                    j = mt % 4 - 2 + 2 * (mt // 4)
                    nc.vector.tensor_scalar_add(
                        ht[:, j, :], hreg(mt), bias[:, 8 + mt:9 + mt]
                    )
                    nc.vector.scalar_tensor_tensor(
                        out=h[:, mt, :], in0=ht[:, j, :], scalar=0.01,
                        in1=ht[:, j, :], op0=mybir.AluOpType.mult,
                        op1=mybir.AluOpType.max,
                    )
                nc.tensor.matmul(
                    ps1[0:1, 0:BC], W2[:, mt, :], h[:, mt, :],
                    start=(i == 0), stop=(i == 7),
                )
            nc.scalar.activation(
                out=sig[:], in_=ps1[0:1, 0:BC], func=AF.Sigmoid,
                bias=b2_val, scale=1.0,
            )
            nc.sync.dma_start(out_d[:], sig[:])

    nc.finalize()
    return nc


def _prep_shared(inputs):
    """Host-side weight prep shared across cores (weights only; all
    input-dependent compute stays on device)."""
    Wd = np.asarray(inputs["W_d"], np.float32)
    bd = np.asarray(inputs["b_d"], np.float32)
    Wc = np.asarray(inputs["W_c"], np.float32)
    bc = np.asarray(inputs["b_c"], np.float32)
    v = np.asarray(inputs["v"], np.float32)[0]          # [CHANNEL, FIELD, EMB]
    lin_w = np.asarray(inputs["lin_w"], np.float32)     # [FIELD, 1]
    lin_b = np.asarray(inputs["lin_b"], np.float32)     # [1]
    W1 = np.asarray(inputs["W1"], np.float32)           # [2041, HID]
    b1 = np.asarray(inputs["b1"], np.float32)
    W2 = np.asarray(inputs["W2"], np.float32)           # [HID, 1]

    # banded FM weights: G[c,f,g] = sum_e v[c,f,e] v[c,g,e]
    G = np.einsum("cfe,cge->cfg", v, v) * GSCALE        # [CHANNEL, 8, 8]
    Gm = np.zeros((128, 7, 8, 128), np.float32)         # [p, d-1, B, m]
    Gs = np.zeros((36, 7, 128), np.float32)             # [6(d-1)+p, B, m]
    m_idx = np.arange(128)
    for d in range(1, 8):
        for Bb in range(8):
            c = 128 * Bb + m_idx                        # [128]
            for f in range(0, 8 - d):
                p = m_idx + f
                ok = (c < CHANNEL) & (p < 128)
                Gm[p[ok], d - 1, Bb, m_idx[ok]] = G[c[ok], f, f + d]
                if Bb < 7:
                    ps_ = p - 128
                    ok2 = (c < CHANNEL) & (ps_ >= 0) & (ps_ < 6)
                    Gs[6 * (d - 1) + ps_[ok2], Bb, m_idx[ok2]] = G[c[ok2], f, f + d]

    # fold the FM linear term (x_fm @ lin_w + lin_b) into W1's top half / b1
    W1a = W1[:NH0].copy()                               # [1024, HID]
    W1b = W1[NH0:]                                      # [CHANNEL, HID]
    for f in range(FIELD):
        W1a[f:f + CHANNEL, :] += lin_w[f, 0] * W1b
    b1e = b1 + lin_b[0] * W1b.sum(0)

    W1a_p = np.zeros((1024, 1024), np.float32)
    W1a_p[:, :HID] = W1a
    W1b_p = np.zeros((1024, 1024), np.float32)
    W1b_p[:CHANNEL, :HID] = W1b * GSCALE                # fm stored /GSCALE
    b1_p = np.zeros(1024, np.float32)
    b1_p[:HID] = b1e
    W2_p = np.zeros(1024, np.float32)
    W2_p[:HID] = W2[:, 0]

    bias_p = np.zeros((128, 16), np.float32)
    bias_p[:, 0:4] = np.ascontiguousarray(bd.reshape(4, 128).T)
    bias_p[:, 4:8] = np.ascontiguousarray(bc.reshape(4, 128).T)
    bias_p[:, 8:16] = np.ascontiguousarray(b1_p.reshape(8, 128).T)

    shared = {
        "Wd": np.ascontiguousarray(
            Wd.reshape(4, 128, NDF).transpose(1, 0, 2)).astype(BF),
        "Wc": np.ascontiguousarray(
            Wc.reshape(2, 128, 2 * NCC).transpose(1, 0, 2)).astype(BF),
        "bias": bias_p,
        "Gm": Gm.astype(E4),
        "Gs": Gs.astype(E4),
        "W1a": np.ascontiguousarray(
            W1a_p.reshape(8, 128, 1024).transpose(1, 0, 2)).astype(BF),
        "W1b": np.ascontiguousarray(
            W1b_p.reshape(8, 128, 1024).transpose(1, 0, 2)).astype(E4),
        "W2": np.ascontiguousarray(W2_p.reshape(8, 128).T)[:, :, None].astype(BF),
    }
    b2_val = float(np.asarray(inputs["b2"], np.float32)[0])
    return shared, b2_val


def _prep_in_maps(inputs):
    dx = np.asarray(inputs["discrete_x"], np.float32)   # [B, NDF]
    cx = np.asarray(inputs["continous_x"], np.float32)  # [B, NCF]
    shared, b2_val = _prep_shared(inputs)

    key = "nc"
    if key not in _cache or _cache.get("b2") != b2_val:
        _cache[key] = _build(b2_val)
        _cache["b2"] = b2_val
    nc = _cache[key]

    in_maps = []
    for i in range(NCORES):
        dxi = dx[i * BC:(i + 1) * BC]                   # [BC, NDF]
        cxi = cx[i * BC:(i + 1) * BC]
        m = dict(shared)
        m["xdT"] = np.ascontiguousarray(
            dxi.T.reshape(4, 128, BC).transpose(1, 0, 2)).astype(BF)
        m["xcT"] = np.ascontiguousarray(
            cxi.T.reshape(2, 128, BC).transpose(1, 0, 2)).astype(BF)
        in_maps.append(m)
    return in_maps, nc


def kernel(**inputs) -> np.ndarray:
    in_maps, nc = _prep_in_maps(inputs)
    res = run_bass_kernel_spmd(nc, in_maps, core_ids=list(range(NCORES)))
    out = np.empty((B, 1), np.float32)
    for i in range(NCORES):
        out[i * BC:(i + 1) * BC, 0] = res.results[i]["out"][0]
    return out


# revision 24
# speedup vs baseline: 1.1145x; 1.1145x over previous
"""Trainium2 Bass kernel for nn_FIN_b: windowed-FM tabular net.

Data-parallel over batch: B=2048 rows split across 8 NeuronCores (256 each).
Activations are feature-major ([feature_partition, batch_free]); every matmul
uses natural-layout weights as the stationary operand.  The windowed FM block
    fm_out[b,c] = 0.5*(sum_e (sum_f x[b,c+f] v[c,f,e])^2 - sum_f x^2 v^2)
is algebraically reduced (f==f' diagonal cancels) to
    fm_out[b,c] = sum_{d=1..7} sum_f D_d[b, c+f] * G[c, f, f+d],
    D_d = x * shift_d(x),  G[c,f,g] = sum_e v[c,f,e] v[c,g,e],
i.e. 7 shifted elementwise products followed by a banded contraction on the
tensor engine with host-precomputed block-banded weights.  The FM linear term
(x_fm @ lin_w) is folded into W1's top half on the host.

Scheduling: ALL bulk data streams on the sync-engine HWDGE ring (measured
the only fast DMA path here: ~230-320 GB/s vs ~20-130 GB/s for the scalar
ring / SWDGE), issued in consumption order with the seven partition-shift
copies spliced into the FIFO between the two W1a halves.  The FM product
path runs in scaled fp8e4m3 (x8/xs/D/G band; G and W1b scaled x64 against
e4m3's subnormal floor), halving both shift traffic and G/W1b weight bytes;
W1a stays bf16 (the x path carries too much signal for fp8; fp8 there
measures 2.2e-2 rel err vs the 2e-2 budget).  Matmuls mix operand dtypes
freely (fp8 stationary x bf16 moving verified exact on HW).  The 8
h-accumulator regions pack 2-per-PSUM-bank (banks pre-zeroed by DVE memset
so every W1 matmul runs start=False: accumulate-onto-zero is correct for
any stale has_written state), freeing 4 banks for the FM reductions, which
interleave with the W1b k-stream on the PE.  A burst of dummy matmuls at
t=0 warms the PE HAM clock gate during the DMA prologue; front relu is
split ACT/DVE and the h-lrelu tail ACT/DVE so no single engine serializes
an epilogue.
"""

import sys

sys.path.insert(0, "/opt/trn_rl_repo")

import numpy as np
import ml_dtypes

import concourse.bass as bass
import concourse.tile as tile
from concourse import bacc, mybir
from concourse.bass_utils import run_bass_kernel_spmd

NDF, NCF, NCC = 512, 256, 256
EMB, FIELD = 16, 8
B = 2048
NH0 = NDF + 2 * NCC          # 1024
CHANNEL = NH0 - FIELD + 1    # 1017
HID = (NH0 + CHANNEL) // 2   # 1020
NCORES = 8
BC = B // NCORES             # 256 batch rows per core

F32 = mybir.dt.float32
BF16 = mybir.dt.bfloat16
FP8 = mybir.dt.float8e4

GSCALE = 64.0                # lifts G (~1e-2) and W1b out of e4m3 subnormals

BF = ml_dtypes.bfloat16
E4 = ml_dtypes.float8_e4m3fn

_cache = {}


def _build(b2_val: float):
    nc = bacc.Bacc()

    xdT_d = nc.dram_tensor("xdT", [128, 4, BC], BF16, kind="ExternalInput")
    xcT_d = nc.dram_tensor("xcT", [128, 2, BC], BF16, kind="ExternalInput")
    Wd_d = nc.dram_tensor("Wd", [128, 4, NDF], BF16, kind="ExternalInput")
    Wc_d = nc.dram_tensor("Wc", [128, 2, 2 * NCC], BF16, kind="ExternalInput")
    bias_d = nc.dram_tensor("bias", [128, 16], F32, kind="ExternalInput")
    Gm_d = nc.dram_tensor("Gm", [128, 7, 8, 128], FP8, kind="ExternalInput")
    Gs_d = nc.dram_tensor("Gs", [36, 7, 128], FP8, kind="ExternalInput")
    W1a_d = nc.dram_tensor("W1a", [128, 8, 1024], BF16, kind="ExternalInput")
    W1b_d = nc.dram_tensor("W1b", [128, 8, 1024], FP8, kind="ExternalInput")
    W2_d = nc.dram_tensor("W2", [128, 8, 1], BF16, kind="ExternalInput")
    out_d = nc.dram_tensor("out", [1, BC], F32, kind="ExternalOutput")

    AF = mybir.ActivationFunctionType

    with tile.TileContext(nc) as tc:
        with (
            tc.tile_pool(name="w", bufs=1) as wp,
            tc.tile_pool(name="act", bufs=1) as ap,
            tc.tile_pool(name="hp", bufs=4, space=bass.MemorySpace.PSUM) as hp,
            tc.tile_pool(name="rp", bufs=4, space=bass.MemorySpace.PSUM) as rp,
        ):
            # ---- weight/input DMAs on the sync HWDGE ring, in
            #      consumption (priority) order ----
            # Everything bulk rides the sync HWDGE ring (the only fast DMA
            # path: ~320 GB/s chunked; the scalar ring and SWDGE measure
            # 5-15x slower).  Issue order = FIFO order = consumption order.
            xdT = wp.tile([128, 4, BC], BF16, tag="xdT")
            nc.sync.dma_start(xdT[:], xdT_d[:])
            Wd = wp.tile([128, 4, NDF], BF16, tag="Wd")
            nc.sync.dma_start(Wd[:], Wd_d[:])
            bias = wp.tile([128, 16], F32, tag="bias")
            nc.sync.dma_start(bias[:], bias_d[:])
            xcT = wp.tile([128, 2, BC], BF16, tag="xcT")
            nc.sync.dma_start(xcT[:], xcT_d[:])
            Wc = wp.tile([128, 2, 2 * NCC], BF16, tag="Wc")
            nc.sync.dma_start(Wc[:], Wc_d[:])
            W1a = wp.tile([128, 8, 1024], BF16, tag="W1a")
            nc.sync.dma_start(W1a[:, 0:2, :], W1a_d[:, 0:2, :])
            Gm = wp.tile([128, 7, 8, 128], FP8, tag="Gm")
            Gs = wp.tile([36, 7, 128], FP8, tag="Gs")
            nc.sync.dma_start(Gs[:], Gs_d[:])
            W2 = wp.tile([128, 8, 1], BF16, tag="W2")
            nc.sync.dma_start(W2[:], W2_d[:])
            W1b = wp.tile([128, 8, 1024], FP8, tag="W1b")

            x = ap.tile([128, 8, BC], BF16, tag="x")
            x8 = ap.tile([128, 9, BC], FP8, tag="x8")   # block 8 stays zero
            xst = [ap.tile([128, 8, BC], FP8, tag=f"xs{d}", name=f"xs{d}")
                   for d in range(1, 8)]
            D = ap.tile([128, 7, 8, BC], FP8, tag="D")
            aux = ap.tile([36, 7, BC], FP8, tag="aux")
            fm = ap.tile([128, 8, BC], BF16, tag="fm")
            h = ap.tile([128, 8, BC], BF16, tag="h")
            sig = ap.tile([1, BC], F32, tag="sig")
            zer = ap.tile([8, 512], BF16, tag="zer")
            with tc.high_priority():
                nc.vector.memset(zer[:], 0.0)
            nc.vector.memset(x8[:, 8, :], 0.0)

            # h accumulators: 8 regions packed 2-per-bank.  Pre-zero the
            # banks so every W1 matmul can run start=False (accumulate onto
            # zeros is exact regardless of stale has_written bits).
            hb = [hp.tile([128, 512], F32, tag="hb", name=f"hb{j}")
                  for j in range(4)]
            for j in range(4):
                nc.vector.memset(hb[j][:], 0.0)

            def hreg(mt):
                j, r = mt % 4, mt // 4
                return hb[j][:, r * BC:(r + 1) * BC]

            # ---- HAM warmup: dummy matmuls keep the PE busy through the
            #      DMA prologue so the real work runs at 2.4 GHz ----
            wps = rp.tile([128, 512], F32, tag="rps", name="warm")
            with tc.high_priority():
                for i in range(8):
                    nc.tensor.matmul(
                        wps[:, 0:512], zer[0:8, 0:128], zer[0:8, 0:512],
                        start=True, stop=True,
                    )

            # ---- front: x = relu([Xd,Xc] @ [Wd,Wc] + b), feature-major ----
            for mt in range(4):
                ps = rp.tile([128, 512], F32, tag="rps", name=f"fpsd{mt}")
                for kt in range(4):
                    nc.tensor.matmul(
                        ps[:, 0:BC], Wd[:, kt, mt * 128:(mt + 1) * 128],
                        xdT[:, kt, :], start=(kt == 0), stop=(kt == 3),
                    )
                if mt % 2 == 0:
                    nc.scalar.activation(
                        out=x[:, mt, :], in_=ps[:, 0:BC], func=AF.Relu,
                        bias=bias[:, mt:mt + 1], scale=1.0,
                    )
                else:
                    nc.vector.tensor_scalar(
                        x[:, mt, :], ps[:, 0:BC], bias[:, mt:mt + 1], 0.0,
                        mybir.AluOpType.add, mybir.AluOpType.max,
                    )
                nc.vector.tensor_copy(x8[:, mt, :], x[:, mt, :])
            for mt in range(4):
                ps = rp.tile([128, 512], F32, tag="rps", name=f"fpsc{mt}")
                for kt in range(2):
                    nc.tensor.matmul(
                        ps[:, 0:BC], Wc[:, kt, mt * 128:(mt + 1) * 128],
                        xcT[:, kt, :], start=(kt == 0), stop=(kt == 1),
                    )
                if mt % 2 == 0:
                    nc.scalar.activation(
                        out=x[:, 4 + mt, :], in_=ps[:, 0:BC], func=AF.Relu,
                        bias=bias[:, 4 + mt:5 + mt], scale=1.0,
                    )
                else:
                    nc.vector.tensor_scalar(
                        x[:, 4 + mt, :], ps[:, 0:BC], bias[:, 4 + mt:5 + mt],
                        0.0, mybir.AluOpType.add, mybir.AluOpType.max,
                    )
                nc.vector.tensor_copy(x8[:, 4 + mt, :], x[:, 4 + mt, :])

            # ---- FM products: D_d = x * shift_d(x) in scaled fp8 ----
            # Shift mains are spliced into the sync FIFO right after W1a's
            # first half; tiny block-boundary pieces go on the scalar ring.
            for d in range(1, 8):
                xs = xst[d - 1]
                nc.sync.dma_start(xs[0:128 - d, :, :], x8[d:128, 0:8, :])
                nc.sync.dma_start(xs[128 - d:128, :, :], x8[0:d, 1:9, :])
                # split each product between DVE and gpsimd
                nc.vector.tensor_mul(
                    D[:, d - 1, 0:5, :], x8[:, 0:5, :], xs[:, 0:5, :]
                )
                nc.gpsimd.tensor_mul(
                    D[:, d - 1, 5:8, :], x8[:, 5:8, :], xs[:, 5:8, :]
                )
            # rest of the weight stream, behind the shifts in the FIFO:
            # Gm (FM needs it first), W1b, then W1a's tail (those h
            # contributions commute, so their matmuls run at the very end)
            nc.sync.dma_start(Gm[:], Gm_d[:])
            for ch in range(2):
                nc.sync.dma_start(
                    W1b[:, 4 * ch:4 * ch + 4, :], W1b_d[:, 4 * ch:4 * ch + 4, :]
                )
            nc.sync.dma_start(W1a[:, 2:5, :], W1a_d[:, 2:5, :])
            nc.sync.dma_start(W1a[:, 5:8, :], W1a_d[:, 5:8, :])
            for d in range(1, 7):
                nc.sync.dma_start(
                    aux[6 * (d - 1):6 * d, :, :], D[0:6, d - 1, 1:8, :]
                )

            # ---- W1a: h += x-part; only kt 0-1 early (the rest of W1a
            #      arrives after the shifts and runs at the end) ----
            for kt in range(2):
                for mt in range(8):
                    nc.tensor.matmul(
                        hreg(mt), W1a[:, kt, mt * 128:(mt + 1) * 128],
                        x[:, kt, :], start=False, stop=False,
                        skip_group_check=True,
                    )

            # ---- FM banded reductions (2 waves x 4 banks), interleaved
            #      with the W1b k-stream as fm blocks complete ----
            def fm_wave(blocks):
                pss = {}
                for Bb in blocks:
                    pss[Bb] = rp.tile([128, 512], F32, tag="rps",
                                      name=f"fmps{Bb}")
                for d in range(1, 8):
                    for Bb in blocks:
                        nc.tensor.matmul(
                            pss[Bb][:, 0:BC], Gm[:, d - 1, Bb, :],
                            D[:, d - 1, Bb, :], start=(d == 1),
                            stop=(d == 7 and Bb == 7),
                        )
                for Bb in blocks:
                    if Bb < 7:
                        nc.tensor.matmul(
                            pss[Bb][:, 0:BC], Gs[:, Bb, :], aux[:, Bb, :],
                            start=False, stop=True,
                        )
                    # fm stored as true_fm/GSCALE (W1b carries the x64)
                    nc.vector.tensor_scalar_mul(
                        fm[:, Bb, :], pss[Bb][:, 0:BC], 1.0 / (GSCALE * GSCALE)
                    )

            def w1b_ktile(kt):
                for mt in range(8):
                    nc.tensor.matmul(
                        hreg(mt), W1b[:, kt, mt * 128:(mt + 1) * 128],
                        fm[:, kt, :], start=False, stop=False,
                        skip_group_check=True,
                    )

            fm_wave([0, 1, 2, 3])
            w1b_ktile(0)
            w1b_ktile(1)
            fm_wave([4, 5, 6, 7])
            for kt in range(2, 8):
                w1b_ktile(kt)
            for kt in range(2, 8):
                mts = [0, 4, 1, 5, 2, 6, 3, 7] if kt == 7 else range(8)
                for mt in mts:
                    nc.tensor.matmul(
                        hreg(mt), W1a[:, kt, mt * 128:(mt + 1) * 128],
                        x[:, kt, :], start=False, stop=(kt == 7),
                        skip_group_check=True,
                    )

            # ---- h = lrelu(hacc + b1); pred = sigmoid(h @ W2 + b2) ----
            ps1 = rp.tile([128, 512], F32, tag="rps", name="ps1")
            ht = ap.tile([128, 4, BC], F32, tag="ht")
            for i, mt in enumerate([0, 4, 1, 5, 2, 6, 3, 7]):
                if mt in (0, 4, 1, 5):
                    nc.scalar.activation(
                        out=h[:, mt, :], in_=hreg(mt), func=AF.Lrelu,
                        bias=bias[:, 8 + mt:9 + mt], scale=1.0, alpha=0.01,
                    )
                else:
                    # DVE lrelu: t = z + b1; h = max(0.01*t, t)
                    j = mt % 4 - 2 + 2 * (mt // 4)
                    nc.vector.tensor_scalar_add(
                        ht[:, j, :], hreg(mt), bias[:, 8 + mt:9 + mt]
                    )
                    nc.vector.scalar_tensor_tensor(
                        out=h[:, mt, :], in0=ht[:, j, :], scalar=0.01,
                        in1=ht[:, j, :], op0=mybir.AluOpType.mult,
                        op1=mybir.AluOpType.max,
                    )
                nc.tensor.matmul(
                    ps1[0:1, 0:BC], W2[:, mt, :], h[:, mt, :],
                    start=(i == 0), stop=(i == 7),
                )
            nc.scalar.activation(
                out=sig[:], in_=ps1[0:1, 0:BC], func=AF.Sigmoid,
                bias=b2_val, scale=1.0,
            )
            nc.sync.dma_start(out_d[:], sig[:])

    nc.finalize()
    return nc


def _prep_shared(inputs):
    """Host-side weight prep shared across cores (weights only; all
    input-dependent compute stays on device)."""
    Wd = np.asarray(inputs["W_d"], np.float32)
    bd = np.asarray(inputs["b_d"], np.float32)
    Wc = np.asarray(inputs["W_c"], np.float32)
    bc = np.asarray(inputs["b_c"], np.float32)
    v = np.asarray(inputs["v"], np.float32)[0]          # [CHANNEL, FIELD, EMB]
    lin_w = np.asarray(inputs["lin_w"], np.float32)     # [FIELD, 1]
    lin_b = np.asarray(inputs["lin_b"], np.float32)     # [1]
    W1 = np.asarray(inputs["W1"], np.float32)           # [2041, HID]
    b1 = np.asarray(inputs["b1"], np.float32)
    W2 = np.asarray(inputs["W2"], np.float32)           # [HID, 1]

    # banded FM weights: G[c,f,g] = sum_e v[c,f,e] v[c,g,e]
    G = np.einsum("cfe,cge->cfg", v, v) * GSCALE        # [CHANNEL, 8, 8]
    Gm = np.zeros((128, 7, 8, 128), np.float32)         # [p, d-1, B, m]
    Gs = np.zeros((36, 7, 128), np.float32)             # [6(d-1)+p, B, m]
    m_idx = np.arange(128)
    for d in range(1, 8):
        for Bb in range(8):
            c = 128 * Bb + m_idx                        # [128]
            for f in range(0, 8 - d):
                p = m_idx + f
                ok = (c < CHANNEL) & (p < 128)
                Gm[p[ok], d - 1, Bb, m_idx[ok]] = G[c[ok], f, f + d]
                if Bb < 7:
                    ps_ = p - 128
                    ok2 = (c < CHANNEL) & (ps_ >= 0) & (ps_ < 6)
                    Gs[6 * (d - 1) + ps_[ok2], Bb, m_idx[ok2]] = G[c[ok2], f, f + d]

    # fold the FM linear term (x_fm @ lin_w + lin_b) into W1's top half / b1
    W1a = W1[:NH0].copy()                               # [1024, HID]
    W1b = W1[NH0:]                                      # [CHANNEL, HID]
    for f in range(FIELD):
        W1a[f:f + CHANNEL, :] += lin_w[f, 0] * W1b
    b1e = b1 + lin_b[0] * W1b.sum(0)

    W1a_p = np.zeros((1024, 1024), np.float32)
    W1a_p[:, :HID] = W1a
    W1b_p = np.zeros((1024, 1024), np.float32)
    W1b_p[:CHANNEL, :HID] = W1b * GSCALE                # fm stored /GSCALE
    b1_p = np.zeros(1024, np.float32)
    b1_p[:HID] = b1e
    W2_p = np.zeros(1024, np.float32)
    W2_p[:HID] = W2[:, 0]

    bias_p = np.zeros((128, 16), np.float32)
    bias_p[:, 0:4] = np.ascontiguousarray(bd.reshape(4, 128).T)
    bias_p[:, 4:8] = np.ascontiguousarray(bc.reshape(4, 128).T)
    bias_p[:, 8:16] = np.ascontiguousarray(b1_p.reshape(8, 128).T)

    shared = {
        "Wd": np.ascontiguousarray(
            Wd.reshape(4, 128, NDF).transpose(1, 0, 2)).astype(BF),
        "Wc": np.ascontiguousarray(
            Wc.reshape(2, 128, 2 * NCC).transpose(1, 0, 2)).astype(BF),
        "bias": bias_p,
        "Gm": Gm.astype(E4),
        "Gs": Gs.astype(E4),
        "W1a": np.ascontiguousarray(
            W1a_p.reshape(8, 128, 1024).transpose(1, 0, 2)).astype(BF),
        "W1b": np.ascontiguousarray(
            W1b_p.reshape(8, 128, 1024).transpose(1, 0, 2)).astype(E4),
        "W2": np.ascontiguousarray(W2_p.reshape(8, 128).T)[:, :, None].astype(BF),
    }
    b2_val = float(np.asarray(inputs["b2"], np.float32)[0])
    return shared, b2_val


def _prep_in_maps(inputs):
    dx = np.asarray(inputs["discrete_x"], np.float32)   # [B, NDF]
    cx = np.asarray(inputs["continous_x"], np.float32)  # [B, NCF]
    shared, b2_val = _prep_shared(inputs)

    key = "nc"
    if key not in _cache or _cache.get("b2") != b2_val:
        _cache[key] = _build(b2_val)
        _cache["b2"] = b2_val
    nc = _cache[key]

    in_maps = []
    for i in range(NCORES):
        dxi = dx[i * BC:(i + 1) * BC]                   # [BC, NDF]
        cxi = cx[i * BC:(i + 1) * BC]
        m = dict(shared)
        m["xdT"] = np.ascontiguousarray(
            dxi.T.reshape(4, 128, BC).transpose(1, 0, 2)).astype(BF)
        m["xcT"] = np.ascontiguousarray(
            cxi.T.reshape(2, 128, BC).transpose(1, 0, 2)).astype(BF)
        in_maps.append(m)
    return in_maps, nc


def kernel(**inputs) -> np.ndarray:
    in_maps, nc = _prep_in_maps(inputs)
    res = run_bass_kernel_spmd(nc, in_maps, core_ids=list(range(NCORES)))
    out = np.empty((B, 1), np.float32)
    for i in range(NCORES):
        out[i * BC:(i + 1) * BC, 0] = res.results[i]["out"][0]
    return out


# revision 26
# speedup vs baseline: 1.1358x; 1.0191x over previous
"""Trainium2 Bass kernel for nn_FIN_b: windowed-FM tabular net.

Data-parallel over batch: B=2048 rows split across 8 NeuronCores (256 each).
Activations are feature-major ([feature_partition, batch_free]); every matmul
uses natural-layout weights as the stationary operand.  The windowed FM block
    fm_out[b,c] = 0.5*(sum_e (sum_f x[b,c+f] v[c,f,e])^2 - sum_f x^2 v^2)
is algebraically reduced (f==f' diagonal cancels) to
    fm_out[b,c] = sum_{d=1..7} sum_f D_d[b, c+f] * G[c, f, f+d],
    D_d = x * shift_d(x),  G[c,f,g] = sum_e v[c,f,e] v[c,g,e],
i.e. 7 shifted elementwise products followed by a banded contraction on the
tensor engine with host-precomputed block-banded weights.  The FM linear term
(x_fm @ lin_w) is folded into W1's top half on the host.

Scheduling: ALL bulk data streams on the sync-engine HWDGE ring (measured
the only fast DMA path here: ~230-320 GB/s vs ~20-130 GB/s for the scalar
ring / SWDGE), issued in consumption order with the seven partition-shift
copies spliced into the FIFO between the two W1a halves.  The FM product
path runs in scaled fp8e4m3 (x8/xs/D/G band; G and W1b scaled x64 against
e4m3's subnormal floor), halving both shift traffic and G/W1b weight bytes;
W1a stays bf16 (the x path carries too much signal for fp8; fp8 there
measures 2.2e-2 rel err vs the 2e-2 budget).  Matmuls mix operand dtypes
freely (fp8 stationary x bf16 moving verified exact on HW).  The 8
h-accumulator regions pack 2-per-PSUM-bank (banks pre-zeroed by DVE memset
so every W1 matmul runs start=False: accumulate-onto-zero is correct for
any stale has_written state), freeing 4 banks for the FM reductions, which
interleave with the W1b k-stream on the PE.  A burst of dummy matmuls at
t=0 warms the PE HAM clock gate during the DMA prologue; front relu is
split ACT/DVE and the h-lrelu tail ACT/DVE so no single engine serializes
an epilogue.
"""

import sys

sys.path.insert(0, "/opt/trn_rl_repo")

import numpy as np
import ml_dtypes

import concourse.bass as bass
import concourse.tile as tile
from concourse import bacc, mybir
from concourse.bass_utils import run_bass_kernel_spmd

NDF, NCF, NCC = 512, 256, 256
EMB, FIELD = 16, 8
B = 2048
NH0 = NDF + 2 * NCC          # 1024
CHANNEL = NH0 - FIELD + 1    # 1017
HID = (NH0 + CHANNEL) // 2   # 1020
NCORES = 8
BC = B // NCORES             # 256 batch rows per core

F32 = mybir.dt.float32
BF16 = mybir.dt.bfloat16
FP8 = mybir.dt.float8e4

GSCALE = 64.0                # lifts G (~1e-2) and W1b out of e4m3 subnormals

BF = ml_dtypes.bfloat16
E4 = ml_dtypes.float8_e4m3fn

_cache = {}


def _build(b2_val: float):
    nc = bacc.Bacc()

    xdT_d = nc.dram_tensor("xdT", [128, 4, BC], BF16, kind="ExternalInput")
    xcT_d = nc.dram_tensor("xcT", [128, 2, BC], BF16, kind="ExternalInput")
    Wd_d = nc.dram_tensor("Wd", [128, 4, NDF], BF16, kind="ExternalInput")
    Wc_d = nc.dram_tensor("Wc", [128, 2, 2 * NCC], BF16, kind="ExternalInput")
    bias_d = nc.dram_tensor("bias", [128, 16], F32, kind="ExternalInput")
    Gm_d = nc.dram_tensor("Gm", [128, 7, 8, 128], FP8, kind="ExternalInput")
    Gs_d = nc.dram_tensor("Gs", [36, 7, 128], FP8, kind="ExternalInput")
    W1a_d = nc.dram_tensor("W1a", [128, 8, 1024], BF16, kind="ExternalInput")
    W1b_d = nc.dram_tensor("W1b", [128, 8, 1024], FP8, kind="ExternalInput")
    W2_d = nc.dram_tensor("W2", [128, 8, 1], BF16, kind="ExternalInput")
    out_d = nc.dram_tensor("out", [1, BC], F32, kind="ExternalOutput")

    AF = mybir.ActivationFunctionType

    with tile.TileContext(nc) as tc:
        with (
            tc.tile_pool(name="w", bufs=1) as wp,
            tc.tile_pool(name="act", bufs=1) as ap,
            tc.tile_pool(name="hp", bufs=4, space=bass.MemorySpace.PSUM) as hp,
            tc.tile_pool(name="rp", bufs=4, space=bass.MemorySpace.PSUM) as rp,
        ):
            # ---- weight/input DMAs on the sync HWDGE ring, in
            #      consumption (priority) order ----
            # Everything bulk rides the sync HWDGE ring (the only fast DMA
            # path: ~320 GB/s chunked; the scalar ring and SWDGE measure
            # 5-15x slower).  Issue order = FIFO order = consumption order.
            xdT = wp.tile([128, 4, BC], BF16, tag="xdT")
            nc.sync.dma_start(xdT[:], xdT_d[:])
            Wd = wp.tile([128, 4, NDF], BF16, tag="Wd")
            nc.sync.dma_start(Wd[:], Wd_d[:])
            bias = wp.tile([128, 16], F32, tag="bias")
            nc.sync.dma_start(bias[:], bias_d[:])
            xcT = wp.tile([128, 2, BC], BF16, tag="xcT")
            nc.sync.dma_start(xcT[:], xcT_d[:])
            Wc = wp.tile([128, 2, 2 * NCC], BF16, tag="Wc")
            nc.sync.dma_start(Wc[:], Wc_d[:])
            W1a = wp.tile([128, 8, 1024], BF16, tag="W1a")
            nc.sync.dma_start(W1a[:, 0:2, :], W1a_d[:, 0:2, :])
            Gm = wp.tile([128, 7, 8, 128], FP8, tag="Gm")
            nc.sync.dma_start(Gm[:], Gm_d[:])
            Gs = wp.tile([36, 7, 128], FP8, tag="Gs")
            nc.sync.dma_start(Gs[:], Gs_d[:])
            W2 = wp.tile([128, 8, 1], BF16, tag="W2")
            nc.sync.dma_start(W2[:], W2_d[:])
            nc.sync.dma_start(W1a[:, 2:4, :], W1a_d[:, 2:4, :])
            W1b = wp.tile([128, 8, 1024], FP8, tag="W1b")

            x = ap.tile([128, 8, BC], BF16, tag="x")
            x8 = ap.tile([128, 9, BC], FP8, tag="x8")   # block 8 stays zero
            xst = [ap.tile([128, 8, BC], FP8, tag=f"xs{d}", name=f"xs{d}")
                   for d in range(1, 8)]
            D = ap.tile([128, 7, 8, BC], FP8, tag="D")
            aux = ap.tile([36, 7, BC], FP8, tag="aux")
            fm = ap.tile([128, 8, BC], BF16, tag="fm")
            h = ap.tile([128, 8, BC], BF16, tag="h")
            sig = ap.tile([1, BC], F32, tag="sig")
            zer = ap.tile([8, 512], BF16, tag="zer")
            with tc.high_priority():
                nc.vector.memset(zer[:], 0.0)
            nc.vector.memset(x8[:, 8, :], 0.0)

            # h accumulators: 8 regions packed 2-per-bank.  Pre-zero the
            # banks so every W1 matmul can run start=False (accumulate onto
            # zeros is exact regardless of stale has_written bits).
            hb = [hp.tile([128, 512], F32, tag="hb", name=f"hb{j}")
                  for j in range(4)]
            for j in range(4):
                nc.vector.memset(hb[j][:], 0.0)

            def hreg(mt):
                j, r = mt % 4, mt // 4
                return hb[j][:, r * BC:(r + 1) * BC]

            # ---- HAM warmup: dummy matmuls keep the PE busy through the
            #      DMA prologue so the real work runs at 2.4 GHz ----
            wps = rp.tile([128, 512], F32, tag="rps", name="warm")
            with tc.high_priority():
                for i in range(8):
                    nc.tensor.matmul(
                        wps[:, 0:512], zer[0:8, 0:128], zer[0:8, 0:512],
                        start=True, stop=True,
                    )

            # ---- front: x = relu([Xd,Xc] @ [Wd,Wc] + b), feature-major ----
            for mt in range(4):
                ps = rp.tile([128, 512], F32, tag="rps", name=f"fpsd{mt}")
                for kt in range(4):
                    nc.tensor.matmul(
                        ps[:, 0:BC], Wd[:, kt, mt * 128:(mt + 1) * 128],
                        xdT[:, kt, :], start=(kt == 0), stop=(kt == 3),
                    )
                if mt % 2 == 0:
                    nc.scalar.activation(
                        out=x[:, mt, :], in_=ps[:, 0:BC], func=AF.Relu,
                        bias=bias[:, mt:mt + 1], scale=1.0,
                    )
                else:
                    nc.vector.tensor_scalar(
                        x[:, mt, :], ps[:, 0:BC], bias[:, mt:mt + 1], 0.0,
                        mybir.AluOpType.add, mybir.AluOpType.max,
                    )
                nc.vector.tensor_copy(x8[:, mt, :], x[:, mt, :])
            for mt in range(4):
                ps = rp.tile([128, 512], F32, tag="rps", name=f"fpsc{mt}")
                for kt in range(2):
                    nc.tensor.matmul(
                        ps[:, 0:BC], Wc[:, kt, mt * 128:(mt + 1) * 128],
                        xcT[:, kt, :], start=(kt == 0), stop=(kt == 1),
                    )
                if mt % 2 == 0:
                    nc.scalar.activation(
                        out=x[:, 4 + mt, :], in_=ps[:, 0:BC], func=AF.Relu,
                        bias=bias[:, 4 + mt:5 + mt], scale=1.0,
                    )
                else:
                    nc.vector.tensor_scalar(
                        x[:, 4 + mt, :], ps[:, 0:BC], bias[:, 4 + mt:5 + mt],
                        0.0, mybir.AluOpType.add, mybir.AluOpType.max,
                    )
                nc.vector.tensor_copy(x8[:, 4 + mt, :], x[:, 4 + mt, :])

            # ---- FM products: D_d = x * shift_d(x) in scaled fp8 ----
            # Shift mains are spliced into the sync FIFO right after W1a's
            # first half; tiny block-boundary pieces go on the scalar ring.
            for d in range(1, 8):
                xs = xst[d - 1]
                nc.sync.dma_start(xs[0:128 - d, :, :], x8[d:128, 0:8, :])
                nc.sync.dma_start(xs[128 - d:128, :, :], x8[0:d, 1:9, :])
                # split each product between DVE and gpsimd
                nc.vector.tensor_mul(
                    D[:, d - 1, 0:5, :], x8[:, 0:5, :], xs[:, 0:5, :]
                )
                nc.gpsimd.tensor_mul(
                    D[:, d - 1, 5:8, :], x8[:, 5:8, :], xs[:, 5:8, :]
                )
            # rest of the weight stream, behind the shifts in the FIFO
            nc.sync.dma_start(W1a[:, 4:8, :], W1a_d[:, 4:8, :])
            for ch in range(2):
                nc.sync.dma_start(
                    W1b[:, 4 * ch:4 * ch + 4, :], W1b_d[:, 4 * ch:4 * ch + 4, :]
                )
            for d in range(1, 7):
                nc.sync.dma_start(
                    aux[6 * (d - 1):6 * d, :, :], D[0:6, d - 1, 1:8, :]
                )

            # ---- W1a: h += x-part, k-tile streamed (overlaps W1a DMA) ----
            for kt in range(8):
                for mt in range(8):
                    nc.tensor.matmul(
                        hreg(mt), W1a[:, kt, mt * 128:(mt + 1) * 128],
                        x[:, kt, :], start=False, stop=False,
                        skip_group_check=True,
                    )

            # ---- FM banded reductions (2 waves x 4 banks), interleaved
            #      with the W1b k-stream as fm blocks complete ----
            def fm_wave(blocks):
                pss = {}
                for Bb in blocks:
                    pss[Bb] = rp.tile([128, 512], F32, tag="rps",
                                      name=f"fmps{Bb}")
                for d in range(1, 8):
                    nc.tensor.matmul(
                        wps[:, 0:512], zer[0:8, 0:128], zer[0:8, 0:512],
                        start=True, stop=True,
                    )
                    for Bb in blocks:
                        nc.tensor.matmul(
                            pss[Bb][:, 0:BC], Gm[:, d - 1, Bb, :],
                            D[:, d - 1, Bb, :], start=(d == 1),
                            stop=(d == 7 and Bb == 7),
                        )
                for Bb in blocks:
                    if Bb < 7:
                        nc.tensor.matmul(
                            pss[Bb][:, 0:BC], Gs[:, Bb, :], aux[:, Bb, :],
                            start=False, stop=True,
                        )
                    # fm stored as true_fm/GSCALE (W1b carries the x64);
                    # copy on ACT so the DVE stays on the product chain
                    nc.scalar.activation(
                        out=fm[:, Bb, :], in_=pss[Bb][:, 0:BC], func=AF.Copy,
                        bias=0.0, scale=1.0 / (GSCALE * GSCALE),
                    )

            def w1b_ktile(kt):
                # final k-tile in bank-pair order so each bank closes early
                mts = [0, 4, 1, 5, 2, 6, 3, 7] if kt == 7 else range(8)
                for mt in mts:
                    nc.tensor.matmul(
                        hreg(mt), W1b[:, kt, mt * 128:(mt + 1) * 128],
                        fm[:, kt, :], start=False, stop=(kt == 7),
                        skip_group_check=True,
                    )

            fm_wave([0, 1, 2, 3])
            w1b_ktile(0)
            w1b_ktile(1)
            fm_wave([4, 5, 6, 7])
            for kt in range(2, 8):
                w1b_ktile(kt)

            # ---- h = lrelu(hacc + b1); pred = sigmoid(h @ W2 + b2) ----
            ps1 = rp.tile([128, 512], F32, tag="rps", name="ps1")
            ht = ap.tile([128, 4, BC], F32, tag="ht")
            for i, mt in enumerate([0, 4, 1, 5, 2, 6, 3, 7]):
                if mt in (0, 4, 1, 5):
                    nc.scalar.activation(
                        out=h[:, mt, :], in_=hreg(mt), func=AF.Lrelu,
                        bias=bias[:, 8 + mt:9 + mt], scale=1.0, alpha=0.01,
                    )
                else:
                    # DVE lrelu: t = z + b1; h = max(0.01*t, t)
                    j = mt % 4 - 2 + 2 * (mt // 4)
                    nc.vector.tensor_scalar_add(
                        ht[:, j, :], hreg(mt), bias[:, 8 + mt:9 + mt]
                    )
                    nc.vector.scalar_tensor_tensor(
                        out=h[:, mt, :], in0=ht[:, j, :], scalar=0.01,
                        in1=ht[:, j, :], op0=mybir.AluOpType.mult,
                        op1=mybir.AluOpType.max,
                    )
                nc.tensor.matmul(
                    ps1[0:1, 0:BC], W2[:, mt, :], h[:, mt, :],
                    start=(i == 0), stop=(i == 7),
                )
            nc.scalar.activation(
                out=sig[:], in_=ps1[0:1, 0:BC], func=AF.Sigmoid,
                bias=b2_val, scale=1.0,
            )
            nc.sync.dma_start(out_d[:], sig[:])

    nc.finalize()
    return nc


def _prep_shared(inputs):
    """Host-side weight prep shared across cores (weights only; all
    input-dependent compute stays on device)."""
    Wd = np.asarray(inputs["W_d"], np.float32)
    bd = np.asarray(inputs["b_d"], np.float32)
    Wc = np.asarray(inputs["W_c"], np.float32)
    bc = np.asarray(inputs["b_c"], np.float32)
    v = np.asarray(inputs["v"], np.float32)[0]          # [CHANNEL, FIELD, EMB]
    lin_w = np.asarray(inputs["lin_w"], np.float32)     # [FIELD, 1]
    lin_b = np.asarray(inputs["lin_b"], np.float32)     # [1]
    W1 = np.asarray(inputs["W1"], np.float32)           # [2041, HID]
    b1 = np.asarray(inputs["b1"], np.float32)
    W2 = np.asarray(inputs["W2"], np.float32)           # [HID, 1]

    # banded FM weights: G[c,f,g] = sum_e v[c,f,e] v[c,g,e]
    G = np.einsum("cfe,cge->cfg", v, v) * GSCALE        # [CHANNEL, 8, 8]
    Gm = np.zeros((128, 7, 8, 128), np.float32)         # [p, d-1, B, m]
    Gs = np.zeros((36, 7, 128), np.float32)             # [6(d-1)+p, B, m]
    m_idx = np.arange(128)
    for d in range(1, 8):
        for Bb in range(8):
            c = 128 * Bb + m_idx                        # [128]
            for f in range(0, 8 - d):
                p = m_idx + f
                ok = (c < CHANNEL) & (p < 128)
                Gm[p[ok], d - 1, Bb, m_idx[ok]] = G[c[ok], f, f + d]
                if Bb < 7:
                    ps_ = p - 128
                    ok2 = (c < CHANNEL) & (ps_ >= 0) & (ps_ < 6)
                    Gs[6 * (d - 1) + ps_[ok2], Bb, m_idx[ok2]] = G[c[ok2], f, f + d]

    # fold the FM linear term (x_fm @ lin_w + lin_b) into W1's top half / b1
    W1a = W1[:NH0].copy()                               # [1024, HID]
    W1b = W1[NH0:]                                      # [CHANNEL, HID]
    for f in range(FIELD):
        W1a[f:f + CHANNEL, :] += lin_w[f, 0] * W1b
    b1e = b1 + lin_b[0] * W1b.sum(0)

    W1a_p = np.zeros((1024, 1024), np.float32)
    W1a_p[:, :HID] = W1a
    W1b_p = np.zeros((1024, 1024), np.float32)
    W1b_p[:CHANNEL, :HID] = W1b * GSCALE                # fm stored /GSCALE
    b1_p = np.zeros(1024, np.float32)
    b1_p[:HID] = b1e
    W2_p = np.zeros(1024, np.float32)
    W2_p[:HID] = W2[:, 0]

    bias_p = np.zeros((128, 16), np.float32)
    bias_p[:, 0:4] = np.ascontiguousarray(bd.reshape(4, 128).T)
    bias_p[:, 4:8] = np.ascontiguousarray(bc.reshape(4, 128).T)
    bias_p[:, 8:16] = np.ascontiguousarray(b1_p.reshape(8, 128).T)

    shared = {
        "Wd": np.ascontiguousarray(
            Wd.reshape(4, 128, NDF).transpose(1, 0, 2)).astype(BF),
        "Wc": np.ascontiguousarray(
            Wc.reshape(2, 128, 2 * NCC).transpose(1, 0, 2)).astype(BF),
        "bias": bias_p,
        "Gm": Gm.astype(E4),
        "Gs": Gs.astype(E4),
        "W1a": np.ascontiguousarray(
            W1a_p.reshape(8, 128, 1024).transpose(1, 0, 2)).astype(BF),
        "W1b": np.ascontiguousarray(
            W1b_p.reshape(8, 128, 1024).transpose(1, 0, 2)).astype(E4),
        "W2": np.ascontiguousarray(W2_p.reshape(8, 128).T)[:, :, None].astype(BF),
    }
    b2_val = float(np.asarray(inputs["b2"], np.float32)[0])
    return shared, b2_val


def _prep_in_maps(inputs):
    dx = np.asarray(inputs["discrete_x"], np.float32)   # [B, NDF]
    cx = np.asarray(inputs["continous_x"], np.float32)  # [B, NCF]
    shared, b2_val = _prep_shared(inputs)

    key = "nc"
    if key not in _cache or _cache.get("b2") != b2_val:
        _cache[key] = _build(b2_val)
        _cache["b2"] = b2_val
    nc = _cache[key]

    in_maps = []
    for i in range(NCORES):
        dxi = dx[i * BC:(i + 1) * BC]                   # [BC, NDF]
        cxi = cx[i * BC:(i + 1) * BC]
        m = dict(shared)
        m["xdT"] = np.ascontiguousarray(
            dxi.T.reshape(4, 128, BC).transpose(1, 0, 2)).astype(BF)
        m["xcT"] = np.ascontiguousarray(
            cxi.T.reshape(2, 128, BC).transpose(1, 0, 2)).astype(BF)
        in_maps.append(m)
    return in_maps, nc


def kernel(**inputs) -> np.ndarray:
    in_maps, nc = _prep_in_maps(inputs)
    res = run_bass_kernel_spmd(nc, in_maps, core_ids=list(range(NCORES)))
    out = np.empty((B, 1), np.float32)
    for i in range(NCORES):
        out[i * BC:(i + 1) * BC, 0] = res.results[i]["out"][0]
    return out
